# revision 26
# baseline (speedup 1.0000x reference)
"""DeepFit kernel for Trainium2: DGCNN edge features + weighted order-2 jet fit.

Full inputs: points [512, 3, 256] f32, weights [512, 256] f32.
Outputs (matching reference): feature [512,10,256,20], beta [512,6],
n_est [512,3], neighbor_normals [512,256,3].

Sharded over 8 NeuronCores by batch (64 patches per core).
"""
import sys
sys.path.insert(0, '/opt/trn_rl_repo')
import numpy as np

B, N, KNN = 512, 256, 20
NCORES = 8
BPC = B // NCORES          # 64 patches per core
NEG_INF = -1.0e30

_cache = {}


def _build(nb):
    """Build + compile the per-core Bass kernel for nb patches (multiple of 8)."""
    import concourse.bacc as bacc
    import concourse.mybir as mybir
    from concourse.tile import TileContext
    from concourse.tile_rust import add_dep_helper
    from contextlib import ExitStack

    F32 = mybir.dt.float32
    U32 = mybir.dt.uint32
    I16 = mybir.dt.int16
    AF = mybir.ActivationFunctionType
    OP = mybir.AluOpType
    AX = mybir.AxisListType

    assert nb % 8 == 0 and nb <= 64
    nocts = nb // 8

    nc = bacc.Bacc("TRN2", target_bir_lowering=False)
    pts_in = nc.dram_tensor("points", [nb, 3, N], F32, kind="ExternalInput")
    w_in = nc.dram_tensor("weights", [nb, N], F32, kind="ExternalInput")
    perm_in = nc.dram_tensor("perm", [128, 20], I16, kind="ExternalInput")
    feat_out = nc.dram_tensor("feature", [nb, 10, N, KNN], F32, kind="ExternalOutput")
    beta_out = nc.dram_tensor("beta", [nb, 6], F32, kind="ExternalOutput")
    nest_out = nc.dram_tensor("n_est", [nb, 3], F32, kind="ExternalOutput")
    nn_out = nc.dram_tensor("neighbor_normals", [nb, N, 3], F32, kind="ExternalOutput")

    with TileContext(nc) as tc:
        ctx = ExitStack()
        const = ctx.enter_context(tc.tile_pool(name="const", bufs=1))
        big = ctx.enter_context(tc.tile_pool(name="big", bufs=1))
        apool = ctx.enter_context(tc.tile_pool(name="apool", bufs=2 * nocts))
        octp = ctx.enter_context(tc.tile_pool(name="octp", bufs=2))
        octe = ctx.enter_context(tc.tile_pool(name="octe", bufs=2))
        perp = ctx.enter_context(tc.tile_pool(name="perp", bufs=3))
        half = ctx.enter_context(tc.tile_pool(name="half", bufs=3))
        gpool = ctx.enter_context(tc.tile_pool(name="gpool", bufs=3))
        solvp = ctx.enter_context(tc.tile_pool(name="solvp", bufs=1))
        dramp = ctx.enter_context(tc.tile_pool(name="dramp", bufs=1, space="DRAM"))
        dramp2 = ctx.enter_context(tc.tile_pool(name="dramp2", bufs=3, space="DRAM"))
        ps_dot = ctx.enter_context(tc.tile_pool(name="ps_dot", bufs=4, space="PSUM"))
        ps_xtx = ctx.enter_context(tc.tile_pool(name="ps_xtx", bufs=1, space="PSUM"))
        ps_cnt = ctx.enter_context(tc.tile_pool(name="ps_cnt", bufs=1, space="PSUM"))
        ps_bc = ctx.enter_context(tc.tile_pool(name="ps_bc", bufs=2, space="PSUM"))

        # ---------------- constants / precompute ----------------
        ones_k1 = const.tile([1, 128], F32, tag="ones")
        nc.vector.memset(ones_k1[:], 1.0)
        ones_col = const.tile([128, 1], F32, tag="onesc")
        nc.vector.memset(ones_col[:], 1.0)
        hconst = const.tile([64, 1], F32, tag="hconst")
        nc.vector.memset(hconst[:], 0.1)
        zeros13 = const.tile([13, 256], F32, tag="zeros13")
        nc.vector.memset(zeros13[:], 0.0)
        perm_t = const.tile([128, 20], I16, tag="perm")
        nc.sync.dma_start(out=perm_t[:], in_=perm_in[:, :])

        # xt_big: partition p = 64*h + q ; free (d:3, n:128)
        xt_big = big.tile([128, 384], F32, tag="xtbig")
        if nb < 64:
            nc.vector.memset(xt_big[:], 0.0)
        for h in range(2):
            nc.sync.dma_start(
                out=xt_big[64 * h:64 * h + nb, :],
                in_=pts_in[:, :, 128 * h:128 * h + 128])
        sq_big = big.tile([128, 384], F32, tag="sqbig")
        nc.vector.tensor_mul(sq_big[:], xt_big[:], xt_big[:])
        xx_big = big.tile([128, 128], F32, tag="xxbig")
        nc.vector.tensor_add(xx_big[:], sq_big[:, 0:128], sq_big[:, 128:256])
        nc.vector.tensor_add(xx_big[:], xx_big[:], sq_big[:, 256:384])
        nxxh_big = big.tile([128, 128], F32, tag="nxxhbig")
        nc.vector.tensor_scalar_mul(nxxh_big[:], xx_big[:], -0.5)
        xx_dram = big.tile([128, 128], F32, tag="xxdram", space="DRAM")
        nc.sync.dma_start(out=xx_dram[:], in_=nxxh_big[:])
        # nxxh_all[0, 256*q + 128*h + n] = -xx[q, 128*h + n]/2
        nxxh_all = big.tile([1, 256 * 64], F32, tag="nxxhall")
        nc.sync.dma_start(
            out=nxxh_all[:, 0:256 * nb],
            in_=xx_dram[:].rearrange("p m -> (p m)")
            .rearrange("(h q m) -> q h m", h=2, m=128)[0:nb, :, :])

        # h preconditioner: means of |x|,|y| over all 256 points
        abs_big = big.tile([128, 384], F32, tag="absbig")
        nc.scalar.activation(abs_big[:], xt_big[:], AF.Abs)
        rsum = big.tile([128, 3], F32, tag="rsum")
        nc.vector.tensor_reduce(
            out=rsum[:], in_=abs_big[:].rearrange("p (d n) -> p d n", d=3),
            op=OP.add, axis=AX.X)
        rsum_hi = big.tile([64, 3], F32, tag="rsumhi")
        nc.sync.dma_start(out=rsum_hi[:], in_=rsum[64:128, :])
        rsum2 = big.tile([64, 3], F32, tag="rsum2")
        nc.vector.tensor_add(rsum2[:], rsum[0:64, :], rsum_hi[:])
        hcol = big.tile([64, 1], F32, tag="hcol")
        mx = big.tile([64, 2], F32, tag="mx")
        nc.vector.tensor_scalar_mul(mx[:], rsum2[:, 0:2], 1.0 / 256.0)
        nc.vector.tensor_add(hcol[:], mx[:, 0:1], mx[:, 1:2])
        nc.vector.tensor_scalar_mul(hcol[:], hcol[:], 0.5)
        hmask = big.tile([64, 1], U32, tag="hmask")
        nc.vector.tensor_scalar(hmask[:], hcol[:], 1e-4, None, op0=OP.is_lt)
        nc.vector.copy_predicated(hcol[:], hmask[:], hconst[:])
        invh = big.tile([64, 1], F32, tag="invh")
        nc.vector.reciprocal(invh[:], hcol[:])

        invh_row = big.tile([1, 64], F32, tag="invhrow")
        nc.sync.dma_start(out=invh_row[:, 0:nb], in_=invh[0:nb, :])
        ps_invh = ps_bc.tile([128, 5 * 64], F32, tag="bcast")
        nc.tensor.matmul(ps_invh[:, 0:nb], ones_k1[:], invh_row[:, 0:nb],
                         start=True, stop=True)
        invh_b = big.tile([128, 64], F32, tag="invhb")
        nc.scalar.activation(invh_b[:, 0:nb], ps_invh[:, 0:nb], AF.Copy)

        xtx_ps = ps_xtx.tile([7, 7 * nb], F32, tag="xtx")
        cnt_ps = ps_cnt.tile([1, 64], F32, tag="cntps")
        dscr = dramp.tile([16, 256], F32, tag="dscr")
        nc.sync.dma_start(out=dscr[3:16, :], in_=zeros13[:])

        oct_store = {}

        # ---------------- main per-oct loop ----------------
        for o in range(nocts):
            oct_half = []
            for h in range(2):
                xt_o = octe.tile([128, 24], F32, tag="xto")
                w_o = octe.tile([128, 8], F32, tag="wo")
                A_o = apool.tile([128, 56], F32, tag="Ao")
                nc.sync.dma_start(
                    out=xt_o[:],
                    in_=pts_in[8 * o:8 * o + 8, :, 128 * h:128 * h + 128]
                    .transpose([2, 0, 1]))
                nc.sync.dma_start(
                    out=w_o[:],
                    in_=w_in[8 * o:8 * o + 8, 128 * h:128 * h + 128]
                    .transpose([1, 0]))
                A3 = A_o[:].rearrange("p (q c) -> p q c", c=7)
                xt3 = xt_o[:].rearrange("p (q c) -> p q c", c=3)
                ivv = invh_b[:, 8 * o:8 * o + 8].unsqueeze(2)
                nc.vector.tensor_mul(A3[:, :, 0:2], xt3[:, :, 0:2],
                                     ivv.broadcast_to([128, 8, 2]))
                nc.vector.tensor_mul(A3[:, :, 2:4], A3[:, :, 0:2], A3[:, :, 0:2])
                nc.vector.tensor_mul(A3[:, :, 4:5], A3[:, :, 0:1], A3[:, :, 1:2])
                nc.vector.memset(A3[:, :, 5:6], 1.0)
                nc.vector.tensor_copy(A3[:, :, 6:7], xt3[:, :, 2:3])
                # valid count
                isgt = octe.tile([128, 8], F32, tag="isgt")
                nc.vector.tensor_scalar(isgt[:], w_o[:], 0.001, None, op0=OP.is_gt)
                nc.tensor.matmul(cnt_ps[:, 8 * o:8 * o + 8], ones_col[:], isgt[:],
                                 start=(h == 0), stop=(h == 1))
                oct_half.append((xt_o, w_o, A_o))
            oct_store[o] = oct_half

            # flags (w_vec select) + broadcast for this oct
            cnt_sb = octe.tile([1, 8], F32, tag="cntsb")
            nc.scalar.activation(cnt_sb[:], cnt_ps[:, 8 * o:8 * o + 8], AF.Copy)
            flag_row = octe.tile([1, 8], F32, tag="flagrow")
            nc.vector.tensor_scalar(flag_row[:], cnt_sb[:], 18.5, None, op0=OP.is_gt)
            ps_flag = ps_bc.tile([128, 5 * 64], F32, tag="bcast")
            nc.tensor.matmul(ps_flag[:, 0:8], ones_k1[:], flag_row[:],
                             start=True, stop=True)
            flag_b = octe.tile([128, 8], F32, tag="flagb")
            nc.scalar.activation(flag_b[:], ps_flag[:, 0:8], AF.Copy)
            om_flag = octe.tile([128, 8], F32, tag="omflag")
            nc.vector.tensor_scalar(om_flag[:], flag_b[:], -1.0, 1.0,
                                    op0=OP.mult, op1=OP.add)
            wA = []
            for h in range(2):
                xt_o, w_o, A_o = oct_half[h]
                wu = octe.tile([128, 8], F32, tag="wu")
                nc.vector.tensor_mul(wu[:], w_o[:], flag_b[:])
                nc.vector.tensor_add(wu[:], wu[:], om_flag[:])
                wA_o = octe.tile([128, 56], F32, tag="wAo")
                nc.vector.tensor_mul(wA_o[:].rearrange("p (q c) -> p q c", c=7),
                                     A_o[:].rearrange("p (q c) -> p q c", c=7),
                                     wu[:].unsqueeze(2).broadcast_to([128, 8, 7]))
                wA.append(wA_o)
            for ql in range(8):
                q = 8 * o + ql
                for h in range(2):
                    nc.tensor.matmul(xtx_ps[:, 7 * q:7 * q + 7],
                                     wA[h][:, 7 * ql:7 * ql + 7],
                                     oct_half[h][2][:, 7 * ql:7 * ql + 7],
                                     start=(h == 0), stop=(h == 1))

            # ---- per-patch: pdist, topk, gather, features ----
            for ql in range(8):
                q = 8 * o + ql
                # gather data/table tile: rows 16g+c = x_c (replicated per group)
                gdat = perp.tile([128, 256], F32, tag="gdat")
                nc.sync.dma_start(out=dscr[0:3, :], in_=pts_in[q, :, :])
                nc.sync.dma_start(
                    out=gdat[:],
                    in_=dscr[:].rearrange("c m -> (c m)").unsqueeze(0)
                    .broadcast_to([8, 4096]))

                for h in range(2):
                    hb = 128 * h
                    xt_o = oct_half[h][0]
                    xt3 = xt_o[:].rearrange("p (q c) -> p q c", c=3)
                    dot_ps = ps_dot.tile([128, 256], F32, tag="dot")
                    nc.tensor.matmul(dot_ps[:], gdat[0:3, hb:hb + 128],
                                     gdat[0:3, :], start=True, stop=False)
                    nc.tensor.matmul(dot_ps[:], ones_k1[:, 0:128],
                                     nxxh_all[:, 256 * q:256 * q + 256],
                                     start=False, stop=True)
                    work = half.tile([128, 256], F32, tag="work")
                    v24 = half.tile([128, 24], F32, tag="v24")
                    i24 = half.tile([128, 24], U32, tag="i24")
                    nc.vector.max(v24[:, 0:8], dot_ps[:])
                    nc.vector.max_index(i24[:, 0:8], v24[:, 0:8], dot_ps[:])
                    nc.vector.match_replace(work[:], v24[:, 0:8], dot_ps[:],
                                            NEG_INF)
                    for r in range(1, 3):
                        nc.vector.max(v24[:, 8 * r:8 * r + 8], work[:])
                        nc.vector.max_index(i24[:, 8 * r:8 * r + 8],
                                            v24[:, 8 * r:8 * r + 8], work[:])
                        if r < 2:
                            nc.vector.match_replace(work[:], v24[:, 8 * r:8 * r + 8],
                                                    work[:], NEG_INF)
                    # idx16 doubles as the wrapped gather index list:
                    # group g = points [hb+16g, hb+16g+16), list pos ii=16j+nloc
                    idx16 = half.tile([128, 20], I16, tag="idx16")
                    nc.gpsimd.tensor_copy(idx16[:], i24[:, 0:20])
                    gmid = gpool.tile([128, 320], F32, tag="gmid")
                    nc.gpsimd.ap_gather(gmid[:], gdat[:], idx16[:], channels=128,
                                        num_elems=256, d=1, num_idxs=320)
                    # permute to ii2 = 20*nloc + j layout (perm is a const list)
                    gout = gpool.tile([128, 320], F32, tag="gout")
                    nc.gpsimd.ap_gather(gout[:], gmid[:], perm_t[:],
                                        channels=128, num_elems=320, d=1,
                                        num_idxs=320)
                    gd = dramp2.tile([128, 320], F32, tag="gd")
                    gdf = gd[:].rearrange("p m -> (p m)")
                    # c-major DRAM layout: addr = 2560*c + 20*r + j
                    nc.sync.dma_start(
                        out=gdf.rearrange("(c g i) -> g c i", c=16, g=8),
                        in_=gout[:])

                    # stag: [d2(20) | diff(60) | ctr(60) | nbr(60)]
                    stag = half.tile([128, 200], F32, tag="stag")
                    nc.scalar.dma_start(
                        out=stag[:, 140:200],
                        in_=gdf.rearrange("(c r j) -> r c j", c=16,
                                          j=20)[:, 0:3, :])
                    nc.scalar.activation(
                        stag[:, 80:140].rearrange("p (c j) -> p c j", c=3),
                        xt3[:, ql:ql + 1, :].squeeze(1).unsqueeze(2)
                        .broadcast_to([128, 3, 20]),
                        AF.Copy)
                    nc.vector.tensor_sub(
                        stag[:, 20:80].rearrange("p (c j) -> p c j", c=3),
                        stag[:, 140:200].rearrange("p (c j) -> p c j", c=3),
                        stag[:, 80:140].rearrange("p (c j) -> p c j", c=3))
                    sqd = half.tile([128, 60], F32, tag="sqd")
                    nc.scalar.activation(sqd[:], stag[:, 20:80], AF.Square)
                    nc.gpsimd.tensor_add(stag[:, 0:20], sqd[:, 0:20], sqd[:, 20:40])
                    nc.gpsimd.tensor_add(stag[:, 0:20], stag[:, 0:20],
                                         sqd[:, 40:60])
                    nc.sync.dma_start(
                        out=feat_out[q:q + 1, 0:10, hb:hb + 128, :]
                        .transpose([0, 2, 1, 3]),
                        in_=stag[:])

        # ---------------- solve (vectorized across patches) ----------------
        xtx_sb = solvp.tile([7, 7 * nb], F32, tag="xtxsb")
        nc.scalar.activation(xtx_sb[:], xtx_ps[:], AF.Copy)
        M = solvp.tile([64, 49], F32, tag="M")
        if nb < 64:
            nc.vector.memset(M[:], 0.0)
            for i in range(7):
                nc.vector.memset(M[:, 7 * i + i:7 * i + i + 1], 1.0)
        for i in range(7):
            nc.sync.dma_start(
                out=M[0:nb, 7 * i:7 * i + 7],
                in_=xtx_sb[i:i + 1, :].rearrange("p (q j) -> p q j", j=7))
        pv = solvp.tile([64, 1], F32, tag="pv")
        f = solvp.tile([64, 1], F32, tag="f")
        t = solvp.tile([64, 7], F32, tag="t")
        for i in range(6):
            nc.vector.reciprocal(pv[:], M[:, 7 * i + i:7 * i + i + 1])
            for r in range(i + 1, 6):
                nc.vector.tensor_mul(f[:], M[:, 7 * r + i:7 * r + i + 1], pv[:])
                w_ = 7 - i
                nc.vector.tensor_scalar_mul(t[:, 0:w_], M[:, 7 * i + i:7 * i + 7],
                                            f[:])
                nc.vector.tensor_sub(M[:, 7 * r + i:7 * r + 7],
                                     M[:, 7 * r + i:7 * r + 7], t[:, 0:w_])
        beta = solvp.tile([64, 6], F32, tag="beta")
        tmp = solvp.tile([64, 1], F32, tag="tmp")
        for i in range(5, -1, -1):
            nc.vector.tensor_copy(tmp[:], M[:, 7 * i + 6:7 * i + 7])
            for j in range(i + 1, 6):
                nc.vector.tensor_mul(f[:], M[:, 7 * i + j:7 * i + j + 1],
                                     beta[:, j:j + 1])
                nc.vector.tensor_sub(tmp[:], tmp[:], f[:])
            nc.vector.reciprocal(pv[:], M[:, 7 * i + i:7 * i + i + 1])
            nc.vector.tensor_mul(beta[:, i:i + 1], tmp[:], pv[:])
        invh2 = solvp.tile([64, 1], F32, tag="invh2")
        nc.vector.tensor_mul(invh2[:], invh[:], invh[:])
        nc.vector.tensor_scalar_mul(beta[:, 0:2], beta[:, 0:2], invh[:])
        nc.vector.tensor_scalar_mul(beta[:, 2:5], beta[:, 2:5], invh2[:])
        nc.sync.dma_start(out=beta_out[:, :], in_=beta[0:nb, :])

        ne = solvp.tile([64, 3], F32, tag="ne")
        nc.vector.tensor_scalar_mul(ne[:, 0:2], beta[:, 0:2], -1.0)
        nc.vector.memset(ne[:, 2:3], 1.0)
        nsq = solvp.tile([64, 2], F32, tag="nsq")
        nc.vector.tensor_mul(nsq[:], ne[:, 0:2], ne[:, 0:2])
        nrm = solvp.tile([64, 1], F32, tag="nrm")
        nc.vector.tensor_add(nrm[:], nsq[:, 0:1], nsq[:, 1:2])
        nc.vector.tensor_scalar(nrm[:], nrm[:], 1.0, None, op0=OP.add)
        nc.scalar.activation(nrm[:], nrm[:], AF.Sqrt)
        nc.vector.tensor_scalar_max(nrm[:], nrm[:], 1e-12)
        rinv = solvp.tile([64, 1], F32, tag="rinv")
        nc.vector.reciprocal(rinv[:], nrm[:])
        nc.vector.tensor_scalar_mul(ne[:], ne[:], rinv[:])
        nc.sync.dma_start(out=nest_out[:, :], in_=ne[0:nb, :])

        # ---------------- neighbor normals ----------------
        coef = solvp.tile([64, 5], F32, tag="coef")
        nc.vector.tensor_copy(coef[:, 0:2], beta[:, 0:2])
        nc.vector.tensor_scalar_mul(coef[:, 2:4], beta[:, 2:4], 2.0)
        nc.vector.tensor_copy(coef[:, 4:5], beta[:, 4:5])
        coef_row = solvp.tile([1, 5 * 64], F32, tag="coefrow")
        nc.sync.dma_start(out=coef_row[:, 0:5 * nb], in_=coef[0:nb, :])
        ps_coef = ps_bc.tile([128, 5 * 64], F32, tag="bcast")
        nc.tensor.matmul(ps_coef[:, 0:5 * nb], ones_k1[:], coef_row[:, 0:5 * nb],
                         start=True, stop=True)
        coef_b = big.tile([128, 5 * 64], F32, tag="coefb")
        nc.scalar.activation(coef_b[:, 0:5 * nb], ps_coef[:, 0:5 * nb], AF.Copy)
        coef3 = coef_b[:].rearrange("p (q k) -> p q k", k=5)

        for o in range(nocts):
            oct_half = oct_store[o]
            for h in range(2):
                xt_o, w_o, A_o = oct_half[h]
                A3 = A_o[:].rearrange("p (q c) -> p q c", c=7)
                xs_v = A3[:, :, 0:1].squeeze(2)
                ys_v = A3[:, :, 1:2].squeeze(2)

                def cf(k):
                    return coef3[:, 8 * o:8 * o + 8, k:k + 1].squeeze(2)

                nx = octp.tile([128, 8], F32, tag="nx")
                ny = octp.tile([128, 8], F32, tag="ny")
                tt = octp.tile([128, 8], F32, tag="tt")
                nn_o = octp.tile([128, 24], F32, tag="nno")
                nc.vector.tensor_mul(nx[:], xs_v, cf(2))
                nc.vector.tensor_add(nx[:], nx[:], cf(0))
                nc.vector.tensor_mul(tt[:], ys_v, cf(4))
                nc.vector.tensor_add(nx[:], nx[:], tt[:])
                nc.vector.tensor_scalar_mul(nx[:], nx[:], -1.0)
                nc.vector.tensor_mul(ny[:], ys_v, cf(3))
                nc.vector.tensor_add(ny[:], ny[:], cf(1))
                nc.vector.tensor_mul(tt[:], xs_v, cf(4))
                nc.vector.tensor_add(ny[:], ny[:], tt[:])
                nc.vector.tensor_scalar_mul(ny[:], ny[:], -1.0)
                s = octp.tile([128, 8], F32, tag="s")
                nc.vector.tensor_mul(s[:], nx[:], nx[:])
                nc.vector.tensor_mul(tt[:], ny[:], ny[:])
                nc.vector.tensor_add(s[:], s[:], tt[:])
                nc.vector.tensor_scalar(s[:], s[:], 1.0, None, op0=OP.add)
                nc.scalar.activation(s[:], s[:], AF.Sqrt)
                nc.vector.tensor_scalar_max(s[:], s[:], 1e-12)
                ri = octp.tile([128, 8], F32, tag="ri")
                nc.vector.reciprocal(ri[:], s[:])
                nn3 = nn_o[:].rearrange("p (q c) -> p q c", c=3)
                nc.vector.tensor_mul(nn3[:, :, 0:1].squeeze(2), nx[:], ri[:])
                nc.vector.tensor_mul(nn3[:, :, 1:2].squeeze(2), ny[:], ri[:])
                nc.vector.tensor_copy(nn3[:, :, 2:3].squeeze(2), ri[:])
                nc.sync.dma_start(
                    out=nn_out[8 * o:8 * o + 8, 128 * h:128 * h + 128, :]
                    .transpose([1, 0, 2]),
                    in_=nn_o[:])
        ctx.close()

    nc.compile()
    return nc


def _perm_array():
    """Const index list for the layout-permute gather.

    Target order ii2 = 20*nloc + j reads gmid position 16*j + nloc.
    Entry for list position ii2 is stored wrapped at
    [16g + ii2 % 16, ii2 // 16] for every group g (same list).
    """
    perm = np.zeros((128, 20), np.int16)
    for ii2 in range(320):
        nloc, j = divmod(ii2, 20)
        v = 16 * j + nloc
        for g in range(8):
            perm[16 * g + ii2 % 16, ii2 // 16] = v
    return perm


def _get_nc(nb):
    if nb not in _cache:
        _cache[nb] = _build(nb)
    return _cache[nb]


def kernel(points, weights):
    from concourse.bass_utils import run_bass_kernel_spmd

    points = np.ascontiguousarray(points, dtype=np.float32)
    weights = np.ascontiguousarray(weights, dtype=np.float32)
    nc = _get_nc(BPC)

    perm = _perm_array()
    in_maps = []
    for c in range(NCORES):
        sl = slice(c * BPC, (c + 1) * BPC)
        in_maps.append({"points": points[sl], "weights": weights[sl],
                        "perm": perm})
    res = run_bass_kernel_spmd(nc, in_maps, core_ids=list(range(NCORES)))

    feature = np.concatenate([r["feature"] for r in res.results], axis=0)
    beta = np.concatenate([r["beta"] for r in res.results], axis=0)
    n_est = np.concatenate([r["n_est"] for r in res.results], axis=0)
    nn = np.concatenate([r["neighbor_normals"] for r in res.results], axis=0)
    return feature, beta, n_est, nn


# revision 28
# speedup vs baseline: 1.3317x; 1.3317x over previous
"""DeepFit kernel for Trainium2: DGCNN edge features + weighted order-2 jet fit.

Full inputs: points [512, 3, 256] f32, weights [512, 256] f32.
Outputs (matching reference): feature [512,10,256,20], beta [512,6],
n_est [512,3], neighbor_normals [512,256,3].

Sharded over 8 NeuronCores by batch (64 patches per core).
"""
import sys
sys.path.insert(0, '/opt/trn_rl_repo')
import numpy as np

B, N, KNN = 512, 256, 20
NCORES = 8
BPC = B // NCORES          # 64 patches per core
NEG_INF = -1.0e30

_cache = {}


def _build(nb):
    """Build + compile the per-core Bass kernel for nb patches (multiple of 8)."""
    import concourse.bacc as bacc
    import concourse.mybir as mybir
    from concourse.tile import TileContext
    from concourse.tile_rust import add_dep_helper
    from contextlib import ExitStack

    F32 = mybir.dt.float32
    U32 = mybir.dt.uint32
    I16 = mybir.dt.int16
    AF = mybir.ActivationFunctionType
    OP = mybir.AluOpType
    AX = mybir.AxisListType

    assert nb % 8 == 0 and nb <= 64
    nocts = nb // 8

    nc = bacc.Bacc("TRN2", target_bir_lowering=False)
    pts_in = nc.dram_tensor("points", [nb, 3, N], F32, kind="ExternalInput")
    w_in = nc.dram_tensor("weights", [nb, N], F32, kind="ExternalInput")
    perm_in = nc.dram_tensor("perm", [128, 20], I16, kind="ExternalInput")
    feat_out = nc.dram_tensor("feature", [nb, 10, N, KNN], F32, kind="ExternalOutput")
    beta_out = nc.dram_tensor("beta", [nb, 6], F32, kind="ExternalOutput")
    nest_out = nc.dram_tensor("n_est", [nb, 3], F32, kind="ExternalOutput")
    nn_out = nc.dram_tensor("neighbor_normals", [nb, N, 3], F32, kind="ExternalOutput")

    with TileContext(nc) as tc:
        ctx = ExitStack()
        const = ctx.enter_context(tc.tile_pool(name="const", bufs=1))
        big = ctx.enter_context(tc.tile_pool(name="big", bufs=1))
        apool = ctx.enter_context(tc.tile_pool(name="apool", bufs=2 * nocts))
        octp = ctx.enter_context(tc.tile_pool(name="octp", bufs=2))
        octe = ctx.enter_context(tc.tile_pool(name="octe", bufs=2))
        perp = ctx.enter_context(tc.tile_pool(name="perp", bufs=4))
        half = ctx.enter_context(tc.tile_pool(name="half", bufs=6))
        gpool = ctx.enter_context(tc.tile_pool(name="gpool", bufs=6))
        solvp = ctx.enter_context(tc.tile_pool(name="solvp", bufs=1))
        dramp = ctx.enter_context(tc.tile_pool(name="dramp", bufs=1, space="DRAM"))
        dramp2 = ctx.enter_context(tc.tile_pool(name="dramp2", bufs=6, space="DRAM"))
        ps_dot = ctx.enter_context(tc.tile_pool(name="ps_dot", bufs=4, space="PSUM"))
        ps_xtx = ctx.enter_context(tc.tile_pool(name="ps_xtx", bufs=1, space="PSUM"))
        ps_cnt = ctx.enter_context(tc.tile_pool(name="ps_cnt", bufs=1, space="PSUM"))
        ps_bc = ctx.enter_context(tc.tile_pool(name="ps_bc", bufs=2, space="PSUM"))

        # ---------------- constants / precompute ----------------
        ones_k1 = const.tile([1, 128], F32, tag="ones")
        nc.vector.memset(ones_k1[:], 1.0)
        ones_col = const.tile([128, 1], F32, tag="onesc")
        nc.vector.memset(ones_col[:], 1.0)
        hconst = const.tile([64, 1], F32, tag="hconst")
        nc.vector.memset(hconst[:], 0.1)
        zeros13 = const.tile([13, 256], F32, tag="zeros13")
        nc.vector.memset(zeros13[:], 0.0)
        perm_t = const.tile([128, 20], I16, tag="perm")
        nc.sync.dma_start(out=perm_t[:], in_=perm_in[:, :])

        # xt_big: partition p = 64*h + q ; free (d:3, n:128)
        xt_big = big.tile([128, 384], F32, tag="xtbig")
        if nb < 64:
            nc.vector.memset(xt_big[:], 0.0)
        for h in range(2):
            nc.sync.dma_start(
                out=xt_big[64 * h:64 * h + nb, :],
                in_=pts_in[:, :, 128 * h:128 * h + 128])
        sq_big = big.tile([128, 384], F32, tag="sqbig")
        nc.vector.tensor_mul(sq_big[:], xt_big[:], xt_big[:])
        xx_big = big.tile([128, 128], F32, tag="xxbig")
        nc.vector.tensor_add(xx_big[:], sq_big[:, 0:128], sq_big[:, 128:256])
        nc.vector.tensor_add(xx_big[:], xx_big[:], sq_big[:, 256:384])
        nxxh_big = big.tile([128, 128], F32, tag="nxxhbig")
        nc.vector.tensor_scalar_mul(nxxh_big[:], xx_big[:], -0.5)
        xx_dram = big.tile([128, 128], F32, tag="xxdram", space="DRAM")
        nc.sync.dma_start(out=xx_dram[:], in_=nxxh_big[:])
        # nxxh_all[0, 256*q + 128*h + n] = -xx[q, 128*h + n]/2
        nxxh_all = big.tile([1, 256 * 64], F32, tag="nxxhall")
        nc.sync.dma_start(
            out=nxxh_all[:, 0:256 * nb],
            in_=xx_dram[:].rearrange("p m -> (p m)")
            .rearrange("(h q m) -> q h m", h=2, m=128)[0:nb, :, :])

        # h preconditioner: means of |x|,|y| over all 256 points
        abs_big = big.tile([128, 384], F32, tag="absbig")
        nc.scalar.activation(abs_big[:], xt_big[:], AF.Abs)
        rsum = big.tile([128, 3], F32, tag="rsum")
        nc.vector.tensor_reduce(
            out=rsum[:], in_=abs_big[:].rearrange("p (d n) -> p d n", d=3),
            op=OP.add, axis=AX.X)
        rsum_hi = big.tile([64, 3], F32, tag="rsumhi")
        nc.sync.dma_start(out=rsum_hi[:], in_=rsum[64:128, :])
        rsum2 = big.tile([64, 3], F32, tag="rsum2")
        nc.vector.tensor_add(rsum2[:], rsum[0:64, :], rsum_hi[:])
        hcol = big.tile([64, 1], F32, tag="hcol")
        mx = big.tile([64, 2], F32, tag="mx")
        nc.vector.tensor_scalar_mul(mx[:], rsum2[:, 0:2], 1.0 / 256.0)
        nc.vector.tensor_add(hcol[:], mx[:, 0:1], mx[:, 1:2])
        nc.vector.tensor_scalar_mul(hcol[:], hcol[:], 0.5)
        hmask = big.tile([64, 1], U32, tag="hmask")
        nc.vector.tensor_scalar(hmask[:], hcol[:], 1e-4, None, op0=OP.is_lt)
        nc.vector.copy_predicated(hcol[:], hmask[:], hconst[:])
        invh = big.tile([64, 1], F32, tag="invh")
        nc.vector.reciprocal(invh[:], hcol[:])

        invh_row = big.tile([1, 64], F32, tag="invhrow")
        nc.sync.dma_start(out=invh_row[:, 0:nb], in_=invh[0:nb, :])
        ps_invh = ps_bc.tile([128, 5 * 64], F32, tag="bcast")
        nc.tensor.matmul(ps_invh[:, 0:nb], ones_k1[:], invh_row[:, 0:nb],
                         start=True, stop=True)
        invh_b = big.tile([128, 64], F32, tag="invhb")
        nc.scalar.activation(invh_b[:, 0:nb], ps_invh[:, 0:nb], AF.Copy)

        xtx_ps = ps_xtx.tile([7, 7 * nb], F32, tag="xtx")
        cnt_ps = ps_cnt.tile([1, 64], F32, tag="cntps")
        dscr_tiles = []
        for i in range(3):
            dt_ = dramp.tile([16, 256], F32, tag=f"dscr{i}", name=f"dscr{i}")
            nc.sync.dma_start(out=dt_[3:16, :], in_=zeros13[:])
            dscr_tiles.append(dt_)

        oct_store = {}

        # ---------------- main per-oct loop ----------------
        for o in range(nocts):
            oct_half = []
            for h in range(2):
                xt_o = octe.tile([128, 24], F32, tag="xto")
                w_o = octe.tile([128, 8], F32, tag="wo")
                A_o = apool.tile([128, 56], F32, tag="Ao")
                nc.sync.dma_start(
                    out=xt_o[:],
                    in_=pts_in[8 * o:8 * o + 8, :, 128 * h:128 * h + 128]
                    .transpose([2, 0, 1]))
                nc.sync.dma_start(
                    out=w_o[:],
                    in_=w_in[8 * o:8 * o + 8, 128 * h:128 * h + 128]
                    .transpose([1, 0]))
                A3 = A_o[:].rearrange("p (q c) -> p q c", c=7)
                xt3 = xt_o[:].rearrange("p (q c) -> p q c", c=3)
                ivv = invh_b[:, 8 * o:8 * o + 8].unsqueeze(2)
                nc.vector.tensor_mul(A3[:, :, 0:2], xt3[:, :, 0:2],
                                     ivv.broadcast_to([128, 8, 2]))
                nc.vector.tensor_mul(A3[:, :, 2:4], A3[:, :, 0:2], A3[:, :, 0:2])
                nc.vector.tensor_mul(A3[:, :, 4:5], A3[:, :, 0:1], A3[:, :, 1:2])
                nc.vector.memset(A3[:, :, 5:6], 1.0)
                nc.vector.tensor_copy(A3[:, :, 6:7], xt3[:, :, 2:3])
                # valid count
                isgt = octe.tile([128, 8], F32, tag="isgt")
                nc.vector.tensor_scalar(isgt[:], w_o[:], 0.001, None, op0=OP.is_gt)
                nc.tensor.matmul(cnt_ps[:, 8 * o:8 * o + 8], ones_col[:], isgt[:],
                                 start=(h == 0), stop=(h == 1))
                oct_half.append((xt_o, w_o, A_o))
            oct_store[o] = oct_half

            # flags (w_vec select) + broadcast for this oct
            cnt_sb = octe.tile([1, 8], F32, tag="cntsb")
            nc.scalar.activation(cnt_sb[:], cnt_ps[:, 8 * o:8 * o + 8], AF.Copy)
            flag_row = octe.tile([1, 8], F32, tag="flagrow")
            nc.vector.tensor_scalar(flag_row[:], cnt_sb[:], 18.5, None, op0=OP.is_gt)
            ps_flag = ps_bc.tile([128, 5 * 64], F32, tag="bcast")
            nc.tensor.matmul(ps_flag[:, 0:8], ones_k1[:], flag_row[:],
                             start=True, stop=True)
            flag_b = octe.tile([128, 8], F32, tag="flagb")
            nc.scalar.activation(flag_b[:], ps_flag[:, 0:8], AF.Copy)
            om_flag = octe.tile([128, 8], F32, tag="omflag")
            nc.vector.tensor_scalar(om_flag[:], flag_b[:], -1.0, 1.0,
                                    op0=OP.mult, op1=OP.add)
            wA = []
            for h in range(2):
                xt_o, w_o, A_o = oct_half[h]
                wu = octe.tile([128, 8], F32, tag="wu")
                nc.vector.tensor_mul(wu[:], w_o[:], flag_b[:])
                nc.vector.tensor_add(wu[:], wu[:], om_flag[:])
                wA_o = octe.tile([128, 56], F32, tag="wAo")
                nc.vector.tensor_mul(wA_o[:].rearrange("p (q c) -> p q c", c=7),
                                     A_o[:].rearrange("p (q c) -> p q c", c=7),
                                     wu[:].unsqueeze(2).broadcast_to([128, 8, 7]))
                wA.append(wA_o)
            for ql in range(8):
                q = 8 * o + ql
                for h in range(2):
                    nc.tensor.matmul(xtx_ps[:, 7 * q:7 * q + 7],
                                     wA[h][:, 7 * ql:7 * ql + 7],
                                     oct_half[h][2][:, 7 * ql:7 * ql + 7],
                                     start=(h == 0), stop=(h == 1))

            # ---- per-patch: pdist, topk, gather, features ----
            for ql in range(8):
                q = 8 * o + ql
                # gather data/table tile: rows 16g+c = x_c (replicated per group)
                gdat = perp.tile([128, 256], F32, tag="gdat")
                dscr = dscr_tiles[q % 3]
                nc.sync.dma_start(out=dscr[0:3, :], in_=pts_in[q, :, :])
                nc.sync.dma_start(
                    out=gdat[:],
                    in_=dscr[:].rearrange("c m -> (c m)").unsqueeze(0)
                    .broadcast_to([8, 4096]))

                for h in range(2):
                    hb = 128 * h
                    xt_o = oct_half[h][0]
                    xt3 = xt_o[:].rearrange("p (q c) -> p q c", c=3)
                    dot_ps = ps_dot.tile([128, 256], F32, tag="dot")
                    nc.tensor.matmul(dot_ps[:], gdat[0:3, hb:hb + 128],
                                     gdat[0:3, :], start=True, stop=False)
                    nc.tensor.matmul(dot_ps[:], ones_k1[:, 0:128],
                                     nxxh_all[:, 256 * q:256 * q + 256],
                                     start=False, stop=True)
                    work = half.tile([128, 256], F32, tag="work")
                    v24 = half.tile([128, 24], F32, tag="v24")
                    i24 = half.tile([128, 24], U32, tag="i24")
                    nc.vector.max(v24[:, 0:8], dot_ps[:])
                    nc.vector.max_index(i24[:, 0:8], v24[:, 0:8], dot_ps[:])
                    nc.vector.match_replace(work[:], v24[:, 0:8], dot_ps[:],
                                            NEG_INF)
                    for r in range(1, 3):
                        nc.vector.max(v24[:, 8 * r:8 * r + 8], work[:])
                        nc.vector.max_index(i24[:, 8 * r:8 * r + 8],
                                            v24[:, 8 * r:8 * r + 8], work[:])
                        if r < 2:
                            nc.vector.match_replace(work[:], v24[:, 8 * r:8 * r + 8],
                                                    work[:], NEG_INF)
                    # idx16 doubles as the wrapped gather index list:
                    # group g = points [hb+16g, hb+16g+16), list pos ii=16j+nloc
                    idx16 = half.tile([128, 20], I16, tag="idx16")
                    nc.gpsimd.tensor_copy(idx16[:], i24[:, 0:20])
                    gmid = gpool.tile([128, 320], F32, tag="gmid")
                    nc.gpsimd.ap_gather(gmid[:], gdat[:], idx16[:], channels=128,
                                        num_elems=256, d=1, num_idxs=320)
                    # permute to ii2 = 20*nloc + j layout (perm is a const list)
                    gout = gpool.tile([128, 320], F32, tag="gout")
                    nc.gpsimd.ap_gather(gout[:], gmid[:], perm_t[:],
                                        channels=128, num_elems=320, d=1,
                                        num_idxs=320)
                    gd = dramp2.tile([128, 320], F32, tag="gd")
                    gdf = gd[:].rearrange("p m -> (p m)")
                    # c-major DRAM layout: addr = 2560*c + 20*r + j
                    nc.scalar.dma_start(
                        out=gdf.rearrange("(c g i) -> g c i", c=16, g=8),
                        in_=gout[:])

                    # stag: [d2(20) | diff(60) | ctr(60) | nbr(60)]
                    stag = half.tile([128, 200], F32, tag="stag")
                    nc.scalar.dma_start(
                        out=stag[:, 140:200],
                        in_=gdf.rearrange("(c r j) -> r c j", c=16,
                                          j=20)[:, 0:3, :])
                    ctrv = xt3[:, ql:ql + 1, :].squeeze(1).unsqueeze(2) \
                        .broadcast_to([128, 3, 20])
                    nc.vector.tensor_copy(
                        stag[:, 80:140].rearrange("p (c j) -> p c j", c=3), ctrv)
                    nc.vector.tensor_sub(
                        stag[:, 20:80].rearrange("p (c j) -> p c j", c=3),
                        stag[:, 140:200].rearrange("p (c j) -> p c j", c=3),
                        ctrv)
                    sqd = half.tile([128, 60], F32, tag="sqd")
                    nc.scalar.activation(sqd[:], stag[:, 20:80], AF.Square)
                    nc.vector.tensor_add(stag[:, 0:20], sqd[:, 0:20], sqd[:, 20:40])
                    nc.vector.tensor_add(stag[:, 0:20], stag[:, 0:20],
                                         sqd[:, 40:60])
                    nc.sync.dma_start(
                        out=feat_out[q:q + 1, 0:10, hb:hb + 128, :]
                        .transpose([0, 2, 1, 3]),
                        in_=stag[:])

        # ---------------- solve (vectorized across patches) ----------------
        xtx_sb = solvp.tile([7, 7 * nb], F32, tag="xtxsb")
        nc.scalar.activation(xtx_sb[:], xtx_ps[:], AF.Copy)
        M = solvp.tile([64, 49], F32, tag="M")
        if nb < 64:
            nc.vector.memset(M[:], 0.0)
            for i in range(7):
                nc.vector.memset(M[:, 7 * i + i:7 * i + i + 1], 1.0)
        for i in range(7):
            nc.sync.dma_start(
                out=M[0:nb, 7 * i:7 * i + 7],
                in_=xtx_sb[i:i + 1, :].rearrange("p (q j) -> p q j", j=7))
        pv = solvp.tile([64, 1], F32, tag="pv")
        f = solvp.tile([64, 1], F32, tag="f")
        t = solvp.tile([64, 7], F32, tag="t")
        for i in range(6):
            nc.vector.reciprocal(pv[:], M[:, 7 * i + i:7 * i + i + 1])
            for r in range(i + 1, 6):
                nc.vector.tensor_mul(f[:], M[:, 7 * r + i:7 * r + i + 1], pv[:])
                w_ = 7 - i
                nc.vector.tensor_scalar_mul(t[:, 0:w_], M[:, 7 * i + i:7 * i + 7],
                                            f[:])
                nc.vector.tensor_sub(M[:, 7 * r + i:7 * r + 7],
                                     M[:, 7 * r + i:7 * r + 7], t[:, 0:w_])
        beta = solvp.tile([64, 6], F32, tag="beta")
        tmp = solvp.tile([64, 1], F32, tag="tmp")
        for i in range(5, -1, -1):
            nc.vector.tensor_copy(tmp[:], M[:, 7 * i + 6:7 * i + 7])
            for j in range(i + 1, 6):
                nc.vector.tensor_mul(f[:], M[:, 7 * i + j:7 * i + j + 1],
                                     beta[:, j:j + 1])
                nc.vector.tensor_sub(tmp[:], tmp[:], f[:])
            nc.vector.reciprocal(pv[:], M[:, 7 * i + i:7 * i + i + 1])
            nc.vector.tensor_mul(beta[:, i:i + 1], tmp[:], pv[:])
        invh2 = solvp.tile([64, 1], F32, tag="invh2")
        nc.vector.tensor_mul(invh2[:], invh[:], invh[:])
        nc.vector.tensor_scalar_mul(beta[:, 0:2], beta[:, 0:2], invh[:])
        nc.vector.tensor_scalar_mul(beta[:, 2:5], beta[:, 2:5], invh2[:])
        nc.sync.dma_start(out=beta_out[:, :], in_=beta[0:nb, :])

        ne = solvp.tile([64, 3], F32, tag="ne")
        nc.vector.tensor_scalar_mul(ne[:, 0:2], beta[:, 0:2], -1.0)
        nc.vector.memset(ne[:, 2:3], 1.0)
        nsq = solvp.tile([64, 2], F32, tag="nsq")
        nc.vector.tensor_mul(nsq[:], ne[:, 0:2], ne[:, 0:2])
        nrm = solvp.tile([64, 1], F32, tag="nrm")
        nc.vector.tensor_add(nrm[:], nsq[:, 0:1], nsq[:, 1:2])
        nc.vector.tensor_scalar(nrm[:], nrm[:], 1.0, None, op0=OP.add)
        nc.scalar.activation(nrm[:], nrm[:], AF.Sqrt)
        nc.vector.tensor_scalar_max(nrm[:], nrm[:], 1e-12)
        rinv = solvp.tile([64, 1], F32, tag="rinv")
        nc.vector.reciprocal(rinv[:], nrm[:])
        nc.vector.tensor_scalar_mul(ne[:], ne[:], rinv[:])
        nc.sync.dma_start(out=nest_out[:, :], in_=ne[0:nb, :])

        # ---------------- neighbor normals ----------------
        coef = solvp.tile([64, 5], F32, tag="coef")
        nc.vector.tensor_copy(coef[:, 0:2], beta[:, 0:2])
        nc.vector.tensor_scalar_mul(coef[:, 2:4], beta[:, 2:4], 2.0)
        nc.vector.tensor_copy(coef[:, 4:5], beta[:, 4:5])
        coef_row = solvp.tile([1, 5 * 64], F32, tag="coefrow")
        nc.sync.dma_start(out=coef_row[:, 0:5 * nb], in_=coef[0:nb, :])
        ps_coef = ps_bc.tile([128, 5 * 64], F32, tag="bcast")
        nc.tensor.matmul(ps_coef[:, 0:5 * nb], ones_k1[:], coef_row[:, 0:5 * nb],
                         start=True, stop=True)
        coef_b = big.tile([128, 5 * 64], F32, tag="coefb")
        nc.scalar.activation(coef_b[:, 0:5 * nb], ps_coef[:, 0:5 * nb], AF.Copy)
        coef3 = coef_b[:].rearrange("p (q k) -> p q k", k=5)

        for o in range(nocts):
            oct_half = oct_store[o]
            for h in range(2):
                xt_o, w_o, A_o = oct_half[h]
                A3 = A_o[:].rearrange("p (q c) -> p q c", c=7)
                xs_v = A3[:, :, 0:1].squeeze(2)
                ys_v = A3[:, :, 1:2].squeeze(2)

                def cf(k):
                    return coef3[:, 8 * o:8 * o + 8, k:k + 1].squeeze(2)

                nx = octp.tile([128, 8], F32, tag="nx")
                ny = octp.tile([128, 8], F32, tag="ny")
                tt = octp.tile([128, 8], F32, tag="tt")
                nn_o = octp.tile([128, 24], F32, tag="nno")
                nc.vector.tensor_mul(nx[:], xs_v, cf(2))
                nc.vector.tensor_add(nx[:], nx[:], cf(0))
                nc.vector.tensor_mul(tt[:], ys_v, cf(4))
                nc.vector.tensor_add(nx[:], nx[:], tt[:])
                nc.vector.tensor_scalar_mul(nx[:], nx[:], -1.0)
                nc.vector.tensor_mul(ny[:], ys_v, cf(3))
                nc.vector.tensor_add(ny[:], ny[:], cf(1))
                nc.vector.tensor_mul(tt[:], xs_v, cf(4))
                nc.vector.tensor_add(ny[:], ny[:], tt[:])
                nc.vector.tensor_scalar_mul(ny[:], ny[:], -1.0)
                s = octp.tile([128, 8], F32, tag="s")
                nc.vector.tensor_mul(s[:], nx[:], nx[:])
                nc.vector.tensor_mul(tt[:], ny[:], ny[:])
                nc.vector.tensor_add(s[:], s[:], tt[:])
                nc.vector.tensor_scalar(s[:], s[:], 1.0, None, op0=OP.add)
                nc.scalar.activation(s[:], s[:], AF.Sqrt)
                nc.vector.tensor_scalar_max(s[:], s[:], 1e-12)
                ri = octp.tile([128, 8], F32, tag="ri")
                nc.vector.reciprocal(ri[:], s[:])
                nn3 = nn_o[:].rearrange("p (q c) -> p q c", c=3)
                nc.vector.tensor_mul(nn3[:, :, 0:1].squeeze(2), nx[:], ri[:])
                nc.vector.tensor_mul(nn3[:, :, 1:2].squeeze(2), ny[:], ri[:])
                nc.vector.tensor_copy(nn3[:, :, 2:3].squeeze(2), ri[:])
                nc.sync.dma_start(
                    out=nn_out[8 * o:8 * o + 8, 128 * h:128 * h + 128, :]
                    .transpose([1, 0, 2]),
                    in_=nn_o[:])
        ctx.close()

    nc.compile()
    return nc


def _perm_array():
    """Const index list for the layout-permute gather.

    Target order ii2 = 20*nloc + j reads gmid position 16*j + nloc.
    Entry for list position ii2 is stored wrapped at
    [16g + ii2 % 16, ii2 // 16] for every group g (same list).
    """
    perm = np.zeros((128, 20), np.int16)
    for ii2 in range(320):
        nloc, j = divmod(ii2, 20)
        v = 16 * j + nloc
        for g in range(8):
            perm[16 * g + ii2 % 16, ii2 // 16] = v
    return perm


def _get_nc(nb):
    if nb not in _cache:
        _cache[nb] = _build(nb)
    return _cache[nb]


def kernel(points, weights):
    from concourse.bass_utils import run_bass_kernel_spmd

    points = np.ascontiguousarray(points, dtype=np.float32)
    weights = np.ascontiguousarray(weights, dtype=np.float32)
    nc = _get_nc(BPC)

    perm = _perm_array()
    in_maps = []
    for c in range(NCORES):
        sl = slice(c * BPC, (c + 1) * BPC)
        in_maps.append({"points": points[sl], "weights": weights[sl],
                        "perm": perm})
    res = run_bass_kernel_spmd(nc, in_maps, core_ids=list(range(NCORES)))

    feature = np.concatenate([r["feature"] for r in res.results], axis=0)
    beta = np.concatenate([r["beta"] for r in res.results], axis=0)
    n_est = np.concatenate([r["n_est"] for r in res.results], axis=0)
    nn = np.concatenate([r["neighbor_normals"] for r in res.results], axis=0)
    return feature, beta, n_est, nn
